# revision 1
# baseline (speedup 1.0000x reference)
"""Trainium2 Bass kernel for causal self-attention (GPT-J RoPE), 8-way
tensor-parallel over heads.

Contract: kernel(x, W_qkv, W_proj) -> np.ndarray  (full [T, D] output)

Sharding: 16 heads / 8 cores = 2 heads per core. Each core computes its
2 heads' QKV projection, RoPE, causal attention, and its partial
W_proj contribution; the host sums the 8 partial outputs (the TP
all-reduce), which is the unshard step.

Per-core device program (all fp32, matmuls in float32r):
  phase B: qT/kT/vT = W @ xT streamed over d-tiles; RoPE via a pair-swap
           permutation matmul + cos/sin elementwise; v transposed to
           [tk, c] via PE and augmented with a ones column.
  phase C: per 512-wide query block i (flash-style, causal):
           ST[tk, tq] score blocks (both heads row-packed, K=64),
           exp on ScalarE straight from PSUM (scale=1/sqrt(C), no max
           subtraction -- scores are ~N(0,1)), diagonal-block masking via
           gpsimd affine_select, AV matmuls with M=65 (v | ones) so the
           softmax denominator falls out of the same pass, then
           reciprocal + partition_broadcast + fused normalize/evict and
           the W_proj matmul (K=64 per head half).
"""

import math
import sys

if "/opt/trn_rl_repo" not in sys.path:
    sys.path.insert(0, "/opt/trn_rl_repo")

import numpy as np

import concourse.bass as bass  # noqa: F401  (engine namespaces live on nc)
import concourse.mybir as mybir
import concourse.tile as tile
from concourse import bacc
from concourse.bass_utils import run_bass_kernel_spmd
from concourse.masks import make_identity

F32 = mybir.dt.float32
F32R = mybir.dt.float32r

N_CORES = 8
N_HEAD = 16
T_FULL = 4096
D_FULL = 1024
C_HEAD = 64


def build_program(T=4096, D=1024, C=64, use_f32r=True, num_devices=8,
                  bufs_pst=2, bufs_po=1, bufs_x=8, bufs_est=6, bufs_out=3,
                  bufs_pp=2, bufs_scr=3, bufs_cs=3, bufs_acc=2, ablate=None):
    """Build the per-core Bass program. C2 = 2 heads * C = 128 partitions."""
    HPC = 2
    C2 = HPC * C
    assert C2 == 128
    TQB = 512                 # query block width
    ND = D // 128             # d-tiles for the QKV contraction
    NT = T // 128             # key tiles
    NI = T // TQB             # query blocks
    JPB = TQB // 128          # key tiles per query block (4)
    NG = max(1, D // TQB)         # proj output chunks
    DW = D // NG                  # proj chunk width (<= 512)
    assert T % TQB == 0 and D % 128 == 0 and DW <= TQB and D % NG == 0

    MMF = F32R if use_f32r else F32   # dtype for matmul-feeding tensors

    def R(ap):
        return ap

    nc = bacc.Bacc(
        "TRN2",
        target_bir_lowering=False,
        debug=False,
        enable_asserts=False,
        num_devices=num_devices,
    )

    xT_d = nc.dram_tensor("xT", [D, T], MMF, kind="ExternalInput").ap()
    wq_d = nc.dram_tensor("wq", [D, C2], MMF, kind="ExternalInput").ap()
    wk_d = nc.dram_tensor("wk", [D, C2], MMF, kind="ExternalInput").ap()
    wv_d = nc.dram_tensor("wv", [D, C2], MMF, kind="ExternalInput").ap()
    perm_d = nc.dram_tensor("perm", [C2, C2], MMF, kind="ExternalInput").ap()
    cos_d = nc.dram_tensor("cosT", [C2, T], F32, kind="ExternalInput").ap()
    sin_d = nc.dram_tensor("sinT", [C2, T], F32, kind="ExternalInput").ap()
    wp0_d = nc.dram_tensor("wp0", [C, D], MMF, kind="ExternalInput").ap()
    wp1_d = nc.dram_tensor("wp1", [C, D], MMF, kind="ExternalInput").ap()
    out_d = nc.dram_tensor("out", [T, D], F32, kind="ExternalOutput").ap()

    scale = 1.0 / math.sqrt(C)

    with tile.TileContext(nc) as tc:
        with (
            tc.tile_pool(name="const", bufs=1) as pconst,
            tc.tile_pool(name="cs", bufs=bufs_cs) as pcs,
            tc.tile_pool(name="xs", bufs=bufs_x) as px,
            tc.tile_pool(name="scr", bufs=bufs_scr) as pscr,
            tc.tile_pool(name="qk", bufs=1) as pqk,
            tc.tile_pool(name="vt", bufs=1) as pv,
            tc.tile_pool(name="est", bufs=bufs_est) as pest,
            tc.tile_pool(name="ot", bufs=1) as pot,
            tc.tile_pool(name="rd", bufs=2) as prd,
            tc.tile_pool(name="outsb", bufs=bufs_out) as pout,
        ):
            # ---- constants ----
            wq_sb, wk_sb, wv_sb = [], [], []
            for d in range(ND):
                for lst, nm, drt in ((wq_sb, "wq", wq_d), (wk_sb, "wk", wk_d),
                                     (wv_sb, "wv", wv_d)):
                    t_ = pconst.tile([128, C2], MMF, tag=f"{nm}{d}", name=f"{nm}{d}")
                    nc.sync.dma_start(t_[:], drt[d * 128:(d + 1) * 128, :])
                    lst.append(t_)
            perm_sb = pconst.tile([C2, C2], MMF, tag="perm", name="perm_sb")
            nc.sync.dma_start(perm_sb[:], perm_d[:])
            ident = pconst.tile([128, 128], F32, tag="ident", name="ident")
            make_identity(nc, ident[:])
            wp0_sb = pconst.tile([C, D], MMF, tag="wp0", name="wp0_sb")
            nc.sync.dma_start(wp0_sb[:], wp0_d[:])
            wp1_sb = pconst.tile([C, D], MMF, tag="wp1", name="wp1_sb")
            nc.sync.dma_start(wp1_sb[:], wp1_d[:])

            # persistent rope'd q/k chunks and v tiles
            qr_t = [pqk.tile([C2, TQB], MMF, tag=f"qr{i}", name=f"qr{i}")
                    for i in range(NI)]
            kr_t = [pqk.tile([C2, TQB], MMF, tag=f"kr{i}", name=f"kr{i}")
                    for i in range(NI)]
            # v tiles [tk, c|1] per head, ones column at col C
            v_t = [[pv.tile([128, C + 1], MMF, tag=f"v{h}_{j}", name=f"v{h}_{j}")
                    for j in range(NT)] for h in range(HPC)]
            ones_col = pconst.tile([128, 1], F32, tag="ones", name="ones_col")
            nc.vector.memset(ones_col[:], 1.0)
            for h in range(HPC):
                for j in range(NT):
                    nc.vector.tensor_copy(v_t[h][j][:, C:C + 1], ones_col[:])
            ot_t = [[pot.tile([C, TQB], MMF, tag=f"ot{h}_{i}", name=f"ot{h}_{i}")
                     for i in range(NI)] for h in range(HPC)]

            # ================= phase B: qkv + rope + v transpose ============
            with (
                tc.tile_pool(name="bacc", bufs=bufs_acc, space="PSUM") as pacc,
                tc.tile_pool(name="brot", bufs=1, space="PSUM") as prot_p,
                tc.tile_pool(name="bvt", bufs=1, space="PSUM") as pvt_p,
            ):
                for i in range(NI):
                    cosc = pcs.tile([C2, TQB], F32, tag="cos", name="cosc")
                    nc.sync.dma_start(cosc[:], cos_d[:, i * TQB:(i + 1) * TQB])
                    sinc = pcs.tile([C2, TQB], F32, tag="sin", name="sinc")
                    nc.sync.dma_start(sinc[:], sin_d[:, i * TQB:(i + 1) * TQB])

                    pq = pacc.tile([C2, TQB], F32, tag="pq", name="pq")
                    pk = pacc.tile([C2, TQB], F32, tag="pk", name="pk")
                    pvp = pacc.tile([C2, TQB], F32, tag="pv", name="pvp")
                    for d in range(ND):
                        xt = px.tile([128, TQB], MMF, tag="xt", name="xt")
                        nc.sync.dma_start(
                            xt[:], xT_d[d * 128:(d + 1) * 128,
                                        i * TQB:(i + 1) * TQB])
                        st, sp = (d == 0), (d == ND - 1)
                        nc.tensor.matmul(pq[:], R(wq_sb[d][:]), R(xt[:]),
                                         start=st, stop=sp)
                        nc.tensor.matmul(pk[:], R(wk_sb[d][:]), R(xt[:]),
                                         start=st, stop=sp)
                        nc.tensor.matmul(pvp[:], R(wv_sb[d][:]), R(xt[:]),
                                         start=st, stop=sp)

                    # rope on q and k
                    for psrc, dst in ((pq, qr_t[i]), (pk, kr_t[i])):
                        raw = pscr.tile([C2, TQB], MMF, tag="raw", name="raw")
                        nc.vector.tensor_copy(raw[:], psrc[:])
                        prot = prot_p.tile([C2, TQB], F32, tag="rot", name="prot")
                        nc.tensor.matmul(prot[:], R(perm_sb[:]), R(raw[:]),
                                         start=True, stop=True)
                        qc = pscr.tile([C2, TQB], F32, tag="qc", name="qc")
                        nc.vector.tensor_mul(qc[:], psrc[:], cosc[:])
                        qs = pscr.tile([C2, TQB], F32, tag="qs", name="qs")
                        nc.vector.tensor_mul(qs[:], prot[:], sinc[:])
                        nc.vector.tensor_add(dst[:], qc[:], qs[:])

                    # v: evict + transpose to [tk, c] per head
                    vraw = pscr.tile([C2, TQB], F32, tag="vraw", name="vraw")
                    nc.vector.tensor_copy(vraw[:], pvp[:])
                    for s in range(JPB):
                        j = i * JPB + s
                        pvt = pvt_p.tile([128, 128], F32, tag="pvt", name="pvt")
                        nc.tensor.transpose(pvt[:], vraw[:, s * 128:(s + 1) * 128],
                                            ident[:])
                        nc.vector.tensor_copy(v_t[0][j][:, 0:C], pvt[:, 0:C])
                        nc.vector.tensor_copy(v_t[1][j][:, 0:C], pvt[:, C:C2])

            # ================= phase C: attention + proj ====================
            with (
                tc.tile_pool(name="pst", bufs=bufs_pst, space="PSUM") as pst_p,
                tc.tile_pool(name="po", bufs=bufs_po, space="PSUM") as po_p,
                tc.tile_pool(name="pp", bufs=bufs_pp, space="PSUM") as pp_p,
            ):
                for i in range(NI if ablate != "B" else 0):
                    po = po_p.tile([128, 2 * TQB], F32, tag="po", name="po")
                    njt = (i + 1) * JPB
                    for jg in range(0, njt, 2):
                        js = [j for j in (jg, jg + 1) if j < njt]
                        los = [max(TQB * i, 128 * j) for j in js]
                        ws = [TQB * (i + 1) - lo for lo in los]
                        offs = list(np.cumsum([0] + ws[:-1]))
                        wtot = int(sum(ws))
                        psts, ests = [], []
                        for h in range(HPC):
                            psts.append(pst_p.tile([128, 2 * TQB], F32,
                                                   tag="pst", name="pst"))
                            ests.append(pest.tile([128, 2 * TQB], MMF,
                                                  tag="est", name="est"))
                        # scores (row-packed across heads)
                        for j, lo, w, o in zip(js, los, ws, offs):
                            jc, jo = divmod(j, JPB)
                            for h in range(HPC):
                                klhs = kr_t[jc][h * C:(h + 1) * C,
                                                jo * 128:(jo + 1) * 128]
                                qrhs = qr_t[i][h * C:(h + 1) * C,
                                               lo - TQB * i:lo - TQB * i + w]
                                nc.tensor.matmul(psts[h][:, o:o + w],
                                                 R(klhs), R(qrhs),
                                                 start=True, stop=True)
                        for h in range(HPC):
                            nc.scalar.activation(ests[h][:, 0:wtot],
                                                 psts[h][:, 0:wtot],
                                                 mybir.ActivationFunctionType.Exp,
                                                 scale=scale)
                            for j, lo, w, o in zip(js, los, ws, offs):
                                if 128 * j >= TQB * i:  # diagonal block
                                    nc.gpsimd.affine_select(
                                        out=ests[h][:, o:o + 128],
                                        in_=ests[h][:, o:o + 128],
                                        compare_op=mybir.AluOpType.is_ge,
                                        fill=0.0, base=0,
                                        pattern=[[1, 128]],
                                        channel_multiplier=-1)
                        # AV with ones column -> O and denominator
                        for j, lo, w, o in zip(js, los, ws, offs):
                            for h in range(HPC):
                                cb = h * TQB + (lo - TQB * i)
                                nc.tensor.matmul(
                                    po[0:C + 1, cb:cb + w],
                                    R(v_t[h][j][:]), R(ests[h][:, o:o + w]),
                                    start=(j == 0), stop=(j == njt - 1),
                                    skip_group_check=True)
                    # normalize: O / denom
                    for h in range(HPC if ablate not in ("AV",) else 0):
                        rd = prd.tile([1, TQB], F32, tag="rd", name="rd")
                        nc.vector.reciprocal(rd[:],
                                             po[C:C + 1, h * TQB:(h + 1) * TQB])
                        rdb = prd.tile([C, TQB], F32, tag="rdb", name="rdb")
                        nc.gpsimd.partition_broadcast(rdb[:], rd[:])
                        nc.vector.tensor_mul(ot_t[h][i][:],
                                             po[0:C, h * TQB:(h + 1) * TQB],
                                             rdb[:])
                    # projection for this block's 4 row-tiles
                    for s in range(JPB if ablate not in ("AV", "NORM") else 0):
                        osb = pout.tile([128, D], F32, tag="osb", name="osb")
                        for g in range(NG):
                            pp = pp_p.tile([128, DW], F32, tag="pp", name="pp")
                            nc.tensor.matmul(
                                pp[:], R(ot_t[0][i][:, s * 128:(s + 1) * 128]),
                                R(wp0_sb[:, g * DW:(g + 1) * DW]),
                                start=True, stop=False)
                            nc.tensor.matmul(
                                pp[:], R(ot_t[1][i][:, s * 128:(s + 1) * 128]),
                                R(wp1_sb[:, g * DW:(g + 1) * DW]),
                                start=False, stop=True)
                            nc.vector.tensor_copy(osb[:, g * DW:(g + 1) * DW],
                                                  pp[:])
                        tt = i * JPB + s
                        nc.sync.dma_start(out_d[tt * 128:(tt + 1) * 128, :],
                                          osb[:])

    nc.compile()
    return nc


def host_inputs(x, W_qkv, W_proj, n_cores=N_CORES):
    """Shard full inputs into per-core input maps."""
    x = np.asarray(x, np.float32)
    W_qkv = np.asarray(W_qkv, np.float32)
    W_proj = np.asarray(W_proj, np.float32)
    T, D = x.shape
    C = C_HEAD
    HPC = (3 * D // 3) // C // n_cores  # heads per core
    H = D // C
    HPC = H // n_cores
    C2 = HPC * C
    Wq, Wk, Wv = W_qkv[0:D], W_qkv[D:2 * D], W_qkv[2 * D:3 * D]

    xT = np.ascontiguousarray(x.T)

    # rope tables [C2, T]
    inv_freq = 1.0 / (10000.0 ** (np.arange(0, C, 2, dtype=np.float64) / C))
    ang = np.arange(T, dtype=np.float64)[None, :] * \
        np.repeat(inv_freq, 2)[:, None]          # [C, T]
    cosT = np.tile(np.cos(ang), (HPC, 1)).astype(np.float32)
    sinT = np.tile(np.sin(ang), (HPC, 1)).astype(np.float32)
    cosT = np.ascontiguousarray(cosT)
    sinT = np.ascontiguousarray(sinT)

    # pair-swap-negate permutation: rot = perm.T @ q  (within each head block)
    perm = np.zeros((C2, C2), np.float32)
    for cp in range(C2):
        if cp % 2 == 0:
            perm[cp + 1, cp] = -1.0
        else:
            perm[cp - 1, cp] = 1.0

    in_maps = []
    for c in range(n_cores):
        rows = slice(c * C2, (c + 1) * C2)
        in_maps.append({
            "xT": xT,
            "wq": np.ascontiguousarray(Wq[rows].T),
            "wk": np.ascontiguousarray(Wk[rows].T),
            "wv": np.ascontiguousarray(Wv[rows].T),
            "perm": perm,
            "cosT": cosT,
            "sinT": sinT,
            "wp0": np.ascontiguousarray(W_proj[:, c * C2:c * C2 + C].T),
            "wp1": np.ascontiguousarray(W_proj[:, c * C2 + C:(c + 1) * C2].T),
        })
    return in_maps


_PROGRAM_CACHE = {}


def _get_program(T, D, use_f32r=True):
    key = (T, D, use_f32r)
    if key not in _PROGRAM_CACHE:
        _PROGRAM_CACHE[key] = build_program(T=T, D=D, use_f32r=use_f32r)
    return _PROGRAM_CACHE[key]


def run_cores(x, W_qkv, W_proj, use_f32r=True, **run_kwargs):
    """Run the SPMD program on 8 cores, return BassKernelResults."""
    nc = _get_program(x.shape[0], x.shape[1], use_f32r)
    in_maps = host_inputs(x, W_qkv, W_proj)
    return run_bass_kernel_spmd(nc, in_maps, core_ids=list(range(N_CORES)),
                                **run_kwargs)


def kernel(x, W_qkv, W_proj):
    res = run_cores(x, W_qkv, W_proj)
    out = np.zeros((x.shape[0], x.shape[1]), np.float32)
    for r in res.results:
        out += r["out"]
    return out



# revision 17
# speedup vs baseline: 1.1576x; 1.1576x over previous
"""Trainium2 Bass kernel for causal self-attention (GPT-J RoPE), 8-way
tensor-parallel over heads.

Contract: kernel(x, W_qkv, W_proj) -> np.ndarray  (full [T, D] output)

Sharding: 16 heads / 8 cores = 2 heads per core. Each core computes its
2 heads' QKV projection, RoPE, causal attention, and its partial
W_proj contribution; the host sums the 8 partial outputs (the TP
all-reduce), which is the unshard step.

v2 design (single fused loop, all matmul operands bf16):
  - Per query block i (512 queries): QKV+RoPE+V-transpose work for
    block i+1 is interleaved as PE filler into block i's attention
    j-loop, so the PE stays busy while the Activation engine (the
    softmax-exp bottleneck) streams.
  - Scores per key tile j: one PSUM tile [128, 1024] holds both heads
    (h0 at cols 0:w, h1 at 512:512+w); one/two exp activations emit
    est (bf16, SBUF); diagonal masking via gpsimd affine_select.
  - AV transposed: po[tq, 65] += est_slice^T @ (v | ones), 65-wide
    bf16 matmuls (half the PE cost of the [c, tq] orientation); the
    ones column yields the softmax denominator for free.
  - Normalize folds the reciprocal into the PSUM->SBUF eviction
    (tensor_scalar per-partition), then a PE transpose re-orients
    O to [c, tq] and both heads stack into one [128, 128] lhsT so the
    W_proj matmul contracts K=128 in a single pass.
  - Engines: ACT = exp only; DVE = all PSUM evictions + PSUM-reading
    muls; Pool = SBUF-only muls/adds/masks; PE = matmuls.
"""

import math
import sys

if "/opt/trn_rl_repo" not in sys.path:
    sys.path.insert(0, "/opt/trn_rl_repo")

import numpy as np
import ml_dtypes

import concourse.bass as bass  # noqa: F401
import concourse.mybir as mybir
import concourse.tile as tile
from concourse import bacc
from concourse.bass_utils import run_bass_kernel_spmd
from concourse.masks import make_identity

F32 = mybir.dt.float32
BF = mybir.dt.bfloat16

N_CORES = 8
N_HEAD = 16
T_FULL = 4096
D_FULL = 1024
C_HEAD = 64


def build_program(T=4096, D=1024, C=64, num_devices=8, dump=False):
    HPC = 2
    C2 = HPC * C            # 128
    TQB = 512
    ND = D // 128           # 8 d-tiles
    NT = T // 128           # 32 key tiles
    NI = T // TQB           # 8 query blocks
    JPB = TQB // 128        # 4

    scale = 1.0 / math.sqrt(C)

    nc = bacc.Bacc(
        "TRN2",
        target_bir_lowering=False,
        debug=False,
        enable_asserts=False,
        num_devices=num_devices,
    )

    xT_d = nc.dram_tensor("xT", [D, T], BF, kind="ExternalInput").ap()
    wq_d = nc.dram_tensor("wq", [D, C2], BF, kind="ExternalInput").ap()
    wk_d = nc.dram_tensor("wk", [D, C2], BF, kind="ExternalInput").ap()
    wv_d = nc.dram_tensor("wv", [D, C2], BF, kind="ExternalInput").ap()
    perm_d = nc.dram_tensor("perm", [C2, C2], BF, kind="ExternalInput").ap()
    cos_d = nc.dram_tensor("cosT", [C2, T], BF, kind="ExternalInput").ap()
    sin_d = nc.dram_tensor("sinT", [C2, T], BF, kind="ExternalInput").ap()
    wp_d = nc.dram_tensor("wp", [C2, D], BF, kind="ExternalInput").ap()
    out_d = nc.dram_tensor("out", [T, D], BF, kind="ExternalOutput").ap()
    if dump:
        qr_dump = nc.dram_tensor("qr_dump", [C2, T], BF,
                                 kind="ExternalOutput").ap()
        kr_dump = nc.dram_tensor("kr_dump", [C2, T], BF,
                                 kind="ExternalOutput").ap()
        v_dump = nc.dram_tensor("v_dump", [T, 2 * (C + 1)], BF,
                                kind="ExternalOutput").ap()
        ot_dump = nc.dram_tensor("ot_dump", [C2, T], BF,
                                 kind="ExternalOutput").ap()
        po_dump = nc.dram_tensor("po_dump", [T, 2 * (C + 1)], F32,
                                 kind="ExternalOutput").ap()
        on_dump = nc.dram_tensor("on_dump", [T, 2 * C], BF,
                                 kind="ExternalOutput").ap()
        est_dump = nc.dram_tensor("est_dump", [8 * 128, 1024], BF,
                                  kind="ExternalOutput").ap()

    with tile.TileContext(nc) as tc:
        with (
            tc.tile_pool(name="const", bufs=1) as pconst,
            tc.tile_pool(name="qk", bufs=1) as pqk,
            tc.tile_pool(name="v2", bufs=1) as pv2,
            tc.tile_pool(name="xs", bufs=12) as px,
            tc.tile_pool(name="cs", bufs=4) as pcs,
            tc.tile_pool(name="raw", bufs=3) as praw,
            tc.tile_pool(name="est", bufs=5) as pest,
            tc.tile_pool(name="on", bufs=4) as pon,
            tc.tile_pool(name="rd", bufs=4) as prd,
            tc.tile_pool(name="otT", bufs=2) as potT,
            tc.tile_pool(name="osb", bufs=3) as posb,
            tc.tile_pool(name="pst", bufs=2, space="PSUM") as pstp,
            tc.tile_pool(name="po", bufs=1, space="PSUM") as ppo,
            tc.tile_pool(name="bps", bufs=1, space="PSUM") as pbps,
        ):
            # ---------------- constants ----------------
            wq_sb, wk_sb, wv_sb = [], [], []
            for d in range(ND):
                for lst, nm, drt in ((wq_sb, "wq", wq_d), (wk_sb, "wk", wk_d),
                                     (wv_sb, "wv", wv_d)):
                    t_ = pconst.tile([128, C2], BF, tag=f"{nm}{d}", name=f"{nm}{d}")
                    nc.sync.dma_start(t_[:], drt[d * 128:(d + 1) * 128, :])
                    lst.append(t_)
            perm_sb = pconst.tile([C2, C2], BF, tag="perm", name="perm_sb")
            nc.sync.dma_start(perm_sb[:], perm_d[:])
            ident = pconst.tile([128, 128], BF, tag="ident", name="ident")
            make_identity(nc, ident[:])
            wp_sb = pconst.tile([C2, D], BF, tag="wp", name="wp_sb")
            nc.sync.dma_start(wp_sb[:], wp_d[:])

            # persistent rope'd q/k [c2, 512] per block, v tiles [t, 130]
            qr_t = [pqk.tile([C2, TQB], BF, tag=f"qr{i}", name=f"qr{i}")
                    for i in range(NI)]
            kr_t = [pqk.tile([C2, TQB], BF, tag=f"kr{i}", name=f"kr{i}")
                    for i in range(NI)]
            v2_t = [pv2.tile([128, 2 * (C + 1)], BF, tag=f"v{j}", name=f"v{j}")
                    for j in range(NT)]
            for j in range(NT):
                nc.gpsimd.memset(v2_t[j][:, C:C + 1], 1.0)
                nc.gpsimd.memset(v2_t[j][:, 2 * C + 1:2 * C + 2], 1.0)

            # ---------------- B(i): qkv + rope + vT units ----------------
            def b_units(ib):
                """Thunk list computing qr[ib], kr[ib], v2[4ib..4ib+3]."""
                st = {}
                units = []

                def u_dma():
                    st["xt"] = []
                    for d in range(ND):
                        xt = px.tile([128, TQB], BF, tag="xt", name="xt")
                        nc.sync.dma_start(
                            xt[:], xT_d[d * 128:(d + 1) * 128,
                                        ib * TQB:(ib + 1) * TQB])
                        st["xt"].append(xt)
                    st["cos"] = pcs.tile([C2, TQB], BF, tag="cos", name="cos")
                    nc.sync.dma_start(st["cos"][:],
                                      cos_d[:, ib * TQB:(ib + 1) * TQB])
                    st["sin"] = pcs.tile([C2, TQB], BF, tag="sin", name="sin")
                    nc.sync.dma_start(st["sin"][:],
                                      sin_d[:, ib * TQB:(ib + 1) * TQB])
                    st["qk"] = pbps.tile([128, 1024], F32, tag="bps", name="bqk")
                units.append(u_dma)

                def u_qk(d):
                    def f():
                        qk = st["qk"]
                        nc.tensor.matmul(qk[:, 0:TQB], wq_sb[d][:],
                                         st["xt"][d][:], start=(d == 0),
                                         stop=(d == ND - 1),
                                         skip_group_check=True)
                        nc.tensor.matmul(qk[:, TQB:2 * TQB], wk_sb[d][:],
                                         st["xt"][d][:], start=(d == 0),
                                         stop=(d == ND - 1),
                                         skip_group_check=True)
                    return f
                for d in range(ND):
                    units.append(u_qk(d))

                def u_evqk():
                    st["rawq"] = praw.tile([C2, TQB], BF, tag="raw", name="rawq")
                    nc.vector.tensor_copy(st["rawq"][:], st["qk"][:, 0:TQB])
                    st["rawk"] = praw.tile([C2, TQB], BF, tag="raw", name="rawk")
                    nc.vector.tensor_copy(st["rawk"][:], st["qk"][:, TQB:2 * TQB])
                    st["rot"] = pbps.tile([128, 1024], F32, tag="bps", name="brot")
                units.append(u_evqk)

                def u_rope(which):
                    def f():
                        raw = st["rawq"] if which == 0 else st["rawk"]
                        dst = qr_t[ib] if which == 0 else kr_t[ib]
                        pr = st["rot"][:, which * TQB:(which + 1) * TQB]
                        nc.tensor.matmul(pr, perm_sb[:], raw[:],
                                         start=True, stop=True,
                                         skip_group_check=True)
                        qc = praw.tile([C2, TQB], BF, tag="qc", name="qc")
                        nc.gpsimd.tensor_mul(qc[:], raw[:], st["cos"][:])
                        qs = praw.tile([C2, TQB], BF, tag="qs", name="qs")
                        nc.vector.tensor_mul(qs[:], pr, st["sin"][:])
                        nc.gpsimd.tensor_add(dst[:], qc[:], qs[:])
                    return f
                units.append(u_rope(0))
                units.append(u_rope(1))

                def u_valloc():
                    st["pv"] = pbps.tile([128, 1024], F32, tag="bps", name="bpv")
                units.append(u_valloc)

                def u_v(d):
                    def f():
                        nc.tensor.matmul(st["pv"][:, 0:TQB], wv_sb[d][:],
                                         st["xt"][d][:], start=(d == 0),
                                         stop=(d == ND - 1),
                                         skip_group_check=True)
                    return f
                for d in range(ND):
                    units.append(u_v(d))

                def u_evv():
                    st["vraw"] = praw.tile([C2, TQB], BF, tag="raw", name="vraw")
                    nc.vector.tensor_copy(st["vraw"][:], st["pv"][:, 0:TQB])
                    st["pvt"] = pbps.tile([128, 1024], F32, tag="bps", name="bpvt")
                units.append(u_evv)

                def u_vt(s):
                    def f():
                        pvtb = st["pvt"][:, s * 64:(s + 1) * 64].bitcast(BF)
                        nc.tensor.transpose(
                            pvtb, st["vraw"][:, s * 128:(s + 1) * 128],
                            ident[:])
                        j = ib * JPB + s
                        nc.vector.tensor_copy(v2_t[j][:, 0:C], pvtb[:, 0:C])
                        nc.vector.tensor_copy(v2_t[j][:, C + 1:2 * C + 1],
                                              pvtb[:, C:2 * C])
                    return f
                for s in range(JPB):
                    units.append(u_vt(s))

                return units

            # ---------------- C(i) units ----------------
            def pocol(h, s):
                return 512 * h + 65 * s

            def c_block(i, filler):
                """Emit attention block i; pop filler thunks between units."""
                njt = (i + 1) * JPB
                po = ppo.tile([128, 1024], F32, tag="po", name="po")
                ests = {}
                widths = {}

                def sc_unit(j):
                    w = min(TQB, TQB * (i + 1) - 128 * j)
                    lo = max(TQB * i, 128 * j)
                    widths[j] = (w, lo)
                    jc, jo = divmod(j, JPB)
                    pst = pstp.tile([128, 1024], F32, tag="pst", name="pst")
                    for h in range(HPC):
                        klhs = kr_t[jc][h * C:(h + 1) * C,
                                        jo * 128:(jo + 1) * 128]
                        qrhs = qr_t[i][h * C:(h + 1) * C,
                                       lo - TQB * i:lo - TQB * i + w]
                        nc.tensor.matmul(pst[:, h * 512:h * 512 + w],
                                         klhs, qrhs, start=True, stop=True,
                                         skip_group_check=True)
                    est = pest.tile([128, 1024], BF, tag="est", name="est")
                    if w == TQB:
                        nc.scalar.activation(est[:, 0:1024], pst[:, 0:1024],
                                             mybir.ActivationFunctionType.Exp,
                                             scale=scale)
                    else:
                        for h in range(HPC):
                            nc.scalar.activation(
                                est[:, h * 512:h * 512 + w],
                                pst[:, h * 512:h * 512 + w],
                                mybir.ActivationFunctionType.Exp, scale=scale)
                    if 128 * j >= TQB * i:  # diagonal tile: mask upper part
                        for h in range(HPC):
                            nc.gpsimd.affine_select(
                                out=est[:, h * 512:h * 512 + 128],
                                in_=est[:, h * 512:h * 512 + 128],
                                compare_op=mybir.AluOpType.is_ge,
                                fill=0.0, base=0, pattern=[[1, 128]],
                                channel_multiplier=-1)
                    ests[j] = est
                    if dump and i == 1:
                        nc.sync.dma_start(
                            est_dump[j * 128:(j + 1) * 128, :], est[:])

                def av_unit(j):
                    w, lo = widths[j]
                    est = ests[j]
                    s_min = max(0, j - JPB * i)
                    for h in range(HPC):
                        for s in range(s_min, JPB):
                            local = TQB * i + 128 * s - lo
                            g = pocol(h, s)
                            # exactly ONE start=True per PSUM bank per round:
                            # it marks the whole 2KB bank pending-zero, so
                            # later first-writes (s>0) see pending bytes and
                            # write fresh rather than accumulate.
                            nc.tensor.matmul(
                                po[:, g:g + C + 1],
                                est[:, h * 512 + local:h * 512 + local + 128],
                                v2_t[j][:, h * (C + 1):(h + 1) * (C + 1)],
                                start=(j == 0 and s == 0),
                                stop=(j == JPB * i + s),
                                skip_group_check=True)

                def tail_unit(s):
                    otT = potT.tile([128, 128], BF, tag="otT", name="otT")
                    # pp doubles as transpose scratch (cols 0:128 f32) before
                    # the proj matmuls overwrite it bank-wide.
                    pp = pstp.tile([128, 1024], F32, tag="pst", name="pp")
                    for h in range(HPC):
                        g = pocol(h, s)
                        rd = prd.tile([128, 1], F32, tag="rd", name="rd")
                        nc.vector.reciprocal(rd[:], po[:, g + C:g + C + 1])
                        on = pon.tile([128, C], BF, tag="on", name="on")
                        nc.vector.tensor_scalar_mul(on[:], po[:, g:g + C], rd[:])
                        if dump:
                            tt_ = i * JPB + s
                            pod = posb.tile([128, C + 1], F32, tag="pod",
                                            name="pod")
                            nc.vector.tensor_copy(pod[:], po[:, g:g + C + 1])
                            nc.sync.dma_start(
                                po_dump[tt_ * 128:(tt_ + 1) * 128,
                                        h * (C + 1):(h + 1) * (C + 1)],
                                pod[:])
                            nc.sync.dma_start(
                                on_dump[tt_ * 128:(tt_ + 1) * 128,
                                        h * C:(h + 1) * C], on[:])
                        scrb = pp[0:C, h * 64:(h + 1) * 64].bitcast(BF)
                        nc.tensor.transpose(scrb, on[:], ident[:])
                        nc.vector.tensor_copy(otT[h * C:(h + 1) * C, :], scrb)
                    for g2 in range(2):
                        nc.tensor.matmul(pp[:, g2 * 512:(g2 + 1) * 512],
                                         otT[:],
                                         wp_sb[:, g2 * 512:(g2 + 1) * 512],
                                         start=True, stop=True,
                                         skip_group_check=True)
                    osb = posb.tile([128, D], BF, tag="osb", name="osb")
                    nc.vector.tensor_copy(osb[:], pp[:])
                    tt = i * JPB + s
                    nc.sync.dma_start(out_d[tt * 128:(tt + 1) * 128, :], osb[:])
                    if dump:
                        nc.sync.dma_start(
                            ot_dump[:, tt * 128:(tt + 1) * 128], otT[:])

                def pop_filler(n):
                    for _ in range(n):
                        if filler:
                            filler.pop(0)()

                rate = (len(filler) + njt - 1) // njt if filler else 0
                for j in range(njt):
                    sc_unit(j)
                    pop_filler(rate)
                    if j >= 1:
                        av_unit(j - 1)
                        if j - 1 >= JPB * i:
                            tail_unit(j - 1 - JPB * i)
                av_unit(njt - 1)
                tail_unit(JPB - 1)
                pop_filler(len(filler))

            # ---------------- main loop ----------------
            for u in b_units(0):
                u()
            for i in range(NI):
                filler = b_units(i + 1) if i + 1 < NI else []
                c_block(i, filler)
            if dump:
                for i in range(NI):
                    nc.sync.dma_start(qr_dump[:, i * TQB:(i + 1) * TQB],
                                      qr_t[i][:])
                    nc.sync.dma_start(kr_dump[:, i * TQB:(i + 1) * TQB],
                                      kr_t[i][:])
                for j in range(NT):
                    nc.sync.dma_start(v_dump[j * 128:(j + 1) * 128, :],
                                      v2_t[j][:])

    nc.compile()
    return nc


def host_inputs(x, W_qkv, W_proj, n_cores=N_CORES):
    """Shard full inputs into per-core input maps (bf16)."""
    x = np.asarray(x, np.float32)
    W_qkv = np.asarray(W_qkv, np.float32)
    W_proj = np.asarray(W_proj, np.float32)
    T, D = x.shape
    C = C_HEAD
    H = D // C
    HPC = H // n_cores
    C2 = HPC * C
    Wq, Wk, Wv = W_qkv[0:D], W_qkv[D:2 * D], W_qkv[2 * D:3 * D]
    bf = ml_dtypes.bfloat16

    xT = np.ascontiguousarray(x.T).astype(bf)

    inv_freq = 1.0 / (10000.0 ** (np.arange(0, C, 2, dtype=np.float64) / C))
    ang = np.arange(T, dtype=np.float64)[None, :] * \
        np.repeat(inv_freq, 2)[:, None]          # [C, T]
    cosT = np.ascontiguousarray(np.tile(np.cos(ang), (HPC, 1))).astype(bf)
    sinT = np.ascontiguousarray(np.tile(np.sin(ang), (HPC, 1))).astype(bf)

    perm = np.zeros((C2, C2), np.float32)
    for cp in range(C2):
        if cp % 2 == 0:
            perm[cp + 1, cp] = -1.0
        else:
            perm[cp - 1, cp] = 1.0
    perm = perm.astype(bf)

    in_maps = []
    for c in range(n_cores):
        rows = slice(c * C2, (c + 1) * C2)
        in_maps.append({
            "xT": xT,
            "wq": np.ascontiguousarray(Wq[rows].T).astype(bf),
            "wk": np.ascontiguousarray(Wk[rows].T).astype(bf),
            "wv": np.ascontiguousarray(Wv[rows].T).astype(bf),
            "perm": perm,
            "cosT": cosT,
            "sinT": sinT,
            "wp": np.ascontiguousarray(W_proj[:, c * C2:(c + 1) * C2].T)
                .astype(bf),
        })
    return in_maps


_PROGRAM_CACHE = {}


def _get_program(T, D, use_f32r=True):
    key = (T, D)
    if key not in _PROGRAM_CACHE:
        _PROGRAM_CACHE[key] = build_program(T=T, D=D)
    return _PROGRAM_CACHE[key]


def run_cores(x, W_qkv, W_proj, **run_kwargs):
    nc = _get_program(x.shape[0], x.shape[1])
    in_maps = host_inputs(x, W_qkv, W_proj)
    return run_bass_kernel_spmd(nc, in_maps, core_ids=list(range(N_CORES)),
                                **run_kwargs)


def kernel(x, W_qkv, W_proj):
    res = run_cores(x, W_qkv, W_proj)
    out = np.zeros((x.shape[0], x.shape[1]), np.float32)
    for r in res.results:
        out += np.asarray(r["out"], np.float32)
    return out


# revision 19
# speedup vs baseline: 1.2269x; 1.0599x over previous
"""Trainium2 Bass kernel for causal self-attention (GPT-J RoPE), 8-way
tensor-parallel over heads.

Contract: kernel(x, W_qkv, W_proj) -> np.ndarray  (full [T, D] output)

Sharding: 16 heads / 8 cores = 2 heads per core. Each core computes its
2 heads' QKV projection, RoPE, causal attention, and its partial
W_proj contribution; the host sums the 8 partial outputs (the TP
all-reduce), which is the unshard step.

v3 design (single fused loop, all matmul operands bf16):
  - Per query block i (512 queries): QKV+RoPE+V-transpose work for
    block i+1 is interleaved as PE filler into block i's attention
    j-loop, so the PE stays busy while the Activation engine (the
    softmax-exp bottleneck) streams. AV matmuls lag the score/exp
    stream by 2 key tiles so they never wait on the exp.
  - Scores per key tile j: one PSUM tile [128, 1024] holds both heads
    (h0 at cols 0:w, h1 at 512:512+w); one/two exp activations emit
    est (bf16, SBUF); diagonal masking via gpsimd affine_select.
  - AV transposed: po[tq, 65] += est_slice^T @ (v | ones), 65-wide
    bf16 matmuls (half the PE cost of the [c, tq] orientation); the
    ones column yields the softmax denominator for free. Exactly one
    start=True per PSUM bank per round (PSUM zeroing is lazy at 2KB
    bank granularity).
  - Tails split: tailA (reciprocal + normalize-evict, frees po early)
    runs inline; tailB (PE transpose, head-stacked K=128 W_proj
    matmul, eviction, output DMA) is deferred into the next block's
    j-loop so its serial cross-engine chain hides under the exp
    stream.
  - Host packs x/weights/cos-sin/perm+wp into 4 DRAM tensors laid out
    so each block needs a single input DMA (50 DMAs total/core).
  - Engines: ACT = exp only; DVE = PSUM evictions + PSUM-reading
    muls; Pool = SBUF-only muls/adds/masks; PE = matmuls.
"""

import math
import sys

if "/opt/trn_rl_repo" not in sys.path:
    sys.path.insert(0, "/opt/trn_rl_repo")

import numpy as np
import ml_dtypes

import concourse.bass as bass  # noqa: F401
import concourse.mybir as mybir
import concourse.tile as tile
from concourse import bacc
from concourse.bass_utils import run_bass_kernel_spmd
from concourse.masks import make_identity

F32 = mybir.dt.float32
BF = mybir.dt.bfloat16

N_CORES = 8
N_HEAD = 16
T_FULL = 4096
D_FULL = 1024
C_HEAD = 64
LAG = 2


def build_program(T=4096, D=1024, C=64, num_devices=8, dump=False):
    HPC = 2
    C2 = HPC * C            # 128
    TQB = 512
    ND = D // 128           # 8 d-tiles
    NT = T // 128           # 32 key tiles
    NI = T // TQB           # 8 query blocks
    JPB = TQB // 128        # 4

    scale = 1.0 / math.sqrt(C)

    nc = bacc.Bacc(
        "TRN2",
        target_bir_lowering=False,
        debug=False,
        enable_asserts=False,
        num_devices=num_devices,
    )

    # xH[p, i*ND*TQB + d*TQB + t] = x[i*TQB+t, d*128+p]
    xH_d = nc.dram_tensor("xH", [128, T * ND], BF, kind="ExternalInput").ap()
    # wH[p, d*3*C2 + {q,k,v}*C2 + c] = W_{q,k,v}[core_c2_c, d*128+p]
    wH_d = nc.dram_tensor("wH", [128, ND * 3 * C2], BF,
                          kind="ExternalInput").ap()
    # csH[p, i*2*TQB + {cos,sin}*TQB + t]
    csH_d = nc.dram_tensor("csH", [C2, NI * 2 * TQB], BF,
                           kind="ExternalInput").ap()
    # pwH = [perm | wp]  ([C2, C2 + D])
    pwH_d = nc.dram_tensor("pwH", [C2, C2 + D], BF, kind="ExternalInput").ap()
    out_d = nc.dram_tensor("out", [T, D], BF, kind="ExternalOutput").ap()
    if dump:
        qr_dump = nc.dram_tensor("qr_dump", [C2, T], BF,
                                 kind="ExternalOutput").ap()
        kr_dump = nc.dram_tensor("kr_dump", [C2, T], BF,
                                 kind="ExternalOutput").ap()
        v_dump = nc.dram_tensor("v_dump", [T, 2 * (C + 1)], BF,
                                kind="ExternalOutput").ap()

    with tile.TileContext(nc) as tc:
        with (
            tc.tile_pool(name="const", bufs=1) as pconst,
            tc.tile_pool(name="qk", bufs=1) as pqk,
            tc.tile_pool(name="v2", bufs=1) as pv2,
            tc.tile_pool(name="xs", bufs=3) as px,
            tc.tile_pool(name="cs", bufs=2) as pcs,
            tc.tile_pool(name="raw", bufs=3) as praw,
            tc.tile_pool(name="est", bufs=5) as pest,
            tc.tile_pool(name="on", bufs=10) as pon,
            tc.tile_pool(name="rd", bufs=10) as prd,
            tc.tile_pool(name="otT", bufs=4) as potT,
            tc.tile_pool(name="osb", bufs=4) as posb,
            tc.tile_pool(name="pst", bufs=2, space="PSUM") as pstp,
            tc.tile_pool(name="po", bufs=1, space="PSUM") as ppo,
            tc.tile_pool(name="bps", bufs=1, space="PSUM") as pbps,
        ):
            # ---------------- constants (2 packed DMAs) ----------------
            wsb = pconst.tile([128, ND * 3 * C2], BF, tag="wsb", name="wsb")
            nc.sync.dma_start(wsb[:], wH_d[:])
            pw_sb = pconst.tile([C2, C2 + D], BF, tag="pw", name="pw_sb")
            nc.sync.dma_start(pw_sb[:], pwH_d[:])

            def w_ap(d, which):   # which: 0=q 1=k 2=v -> [128, C2] lhsT
                o = d * 3 * C2 + which * C2
                return wsb[:, o:o + C2]

            perm_ap = pw_sb[:, 0:C2]
            wp_ap = pw_sb[:, C2:C2 + D]

            ident = pconst.tile([128, 128], BF, tag="ident", name="ident")
            make_identity(nc, ident[:])

            # persistent rope'd q/k [c2, 512] per block, v tiles [t, 130]
            qr_t = [pqk.tile([C2, TQB], BF, tag=f"qr{i}", name=f"qr{i}")
                    for i in range(NI)]
            kr_t = [pqk.tile([C2, TQB], BF, tag=f"kr{i}", name=f"kr{i}")
                    for i in range(NI)]
            v2_t = [pv2.tile([128, 2 * (C + 1)], BF, tag=f"v{j}", name=f"v{j}")
                    for j in range(NT)]
            for j in range(NT):
                nc.gpsimd.memset(v2_t[j][:, C:C + 1], 1.0)
                nc.gpsimd.memset(v2_t[j][:, 2 * C + 1:2 * C + 2], 1.0)

            # ---------------- B(i): qkv + rope + vT units ----------------
            def b_units(ib):
                """Thunk list computing qr[ib], kr[ib], v2[4ib..4ib+3]."""
                st = {}
                units = []

                def u_dma():
                    st["xt"] = px.tile([128, ND * TQB], BF, tag="xt",
                                       name="xt")
                    nc.sync.dma_start(
                        st["xt"][:],
                        xH_d[:, ib * ND * TQB:(ib + 1) * ND * TQB])
                    st["cs"] = pcs.tile([C2, 2 * TQB], BF, tag="cs",
                                        name="cs")
                    nc.sync.dma_start(
                        st["cs"][:],
                        csH_d[:, ib * 2 * TQB:(ib + 1) * 2 * TQB])
                    st["qk"] = pbps.tile([128, 1024], F32, tag="bps",
                                         name="bqk")
                units.append(u_dma)

                def u_qk(d):
                    def f():
                        xs = st["xt"][:, d * TQB:(d + 1) * TQB]
                        nc.tensor.matmul(st["qk"][:, 0:TQB], w_ap(d, 0), xs,
                                         start=(d == 0), stop=(d == ND - 1),
                                         skip_group_check=True)
                        nc.tensor.matmul(st["qk"][:, TQB:2 * TQB], w_ap(d, 1),
                                         xs, start=(d == 0),
                                         stop=(d == ND - 1),
                                         skip_group_check=True)
                    return f
                for d in range(ND):
                    units.append(u_qk(d))

                def u_evqk():
                    st["rawq"] = praw.tile([C2, TQB], BF, tag="raw",
                                           name="rawq")
                    nc.vector.tensor_copy(st["rawq"][:], st["qk"][:, 0:TQB])
                    st["rawk"] = praw.tile([C2, TQB], BF, tag="raw",
                                           name="rawk")
                    nc.vector.tensor_copy(st["rawk"][:],
                                          st["qk"][:, TQB:2 * TQB])
                    st["rot"] = pbps.tile([128, 1024], F32, tag="bps",
                                          name="brot")
                units.append(u_evqk)

                def u_rope(which):
                    def f():
                        raw = st["rawq"] if which == 0 else st["rawk"]
                        dst = qr_t[ib] if which == 0 else kr_t[ib]
                        pr = st["rot"][:, which * TQB:(which + 1) * TQB]
                        nc.tensor.matmul(pr, perm_ap, raw[:],
                                         start=True, stop=True,
                                         skip_group_check=True)
                        qc = praw.tile([C2, TQB], BF, tag="qc", name="qc")
                        nc.gpsimd.tensor_mul(qc[:], raw[:],
                                             st["cs"][:, 0:TQB])
                        qs = praw.tile([C2, TQB], BF, tag="qs", name="qs")
                        nc.vector.tensor_mul(qs[:], pr,
                                             st["cs"][:, TQB:2 * TQB])
                        nc.gpsimd.tensor_add(dst[:], qc[:], qs[:])
                    return f
                units.append(u_rope(0))
                units.append(u_rope(1))

                def u_valloc():
                    st["pv"] = pbps.tile([128, 1024], F32, tag="bps",
                                         name="bpv")
                units.append(u_valloc)

                def u_v(d):
                    def f():
                        nc.tensor.matmul(
                            st["pv"][:, 0:TQB], w_ap(d, 2),
                            st["xt"][:, d * TQB:(d + 1) * TQB],
                            start=(d == 0), stop=(d == ND - 1),
                            skip_group_check=True)
                    return f
                for d in range(ND):
                    units.append(u_v(d))

                def u_evv():
                    st["vraw"] = praw.tile([C2, TQB], BF, tag="raw",
                                           name="vraw")
                    nc.vector.tensor_copy(st["vraw"][:], st["pv"][:, 0:TQB])
                    st["pvt"] = pbps.tile([128, 1024], F32, tag="bps",
                                          name="bpvt")
                units.append(u_evv)

                def u_vt(s):
                    def f():
                        pvtb = st["pvt"][:, s * 64:(s + 1) * 64].bitcast(BF)
                        nc.tensor.transpose(
                            pvtb, st["vraw"][:, s * 128:(s + 1) * 128],
                            ident[:])
                        j = ib * JPB + s
                        nc.vector.tensor_copy(v2_t[j][:, 0:C], pvtb[:, 0:C])
                        nc.vector.tensor_copy(v2_t[j][:, C + 1:2 * C + 1],
                                              pvtb[:, C:2 * C])
                    return f
                for s in range(JPB):
                    units.append(u_vt(s))

                return units

            # ---------------- C(i) ----------------
            def pocol(h, s):
                return 512 * h + 65 * s

            def c_block(i, filler, deferred):
                """Emit attention block i. `filler`: B(i+1) units; `deferred`:
                tailB units of block i-1. Returns this block's tailB units
                (empty if emitted inline for the last block)."""
                njt = (i + 1) * JPB
                po = ppo.tile([128, 1024], F32, tag="po", name="po")
                ests = {}
                widths = {}
                ons = {}

                def sc_unit(j):
                    w = min(TQB, TQB * (i + 1) - 128 * j)
                    lo = max(TQB * i, 128 * j)
                    widths[j] = (w, lo)
                    jc, jo = divmod(j, JPB)
                    pst = pstp.tile([128, 1024], F32, tag="pst", name="pst")
                    for h in range(HPC):
                        klhs = kr_t[jc][h * C:(h + 1) * C,
                                        jo * 128:(jo + 1) * 128]
                        qrhs = qr_t[i][h * C:(h + 1) * C,
                                       lo - TQB * i:lo - TQB * i + w]
                        nc.tensor.matmul(pst[:, h * 512:h * 512 + w],
                                         klhs, qrhs, start=True, stop=True,
                                         skip_group_check=True)
                    est = pest.tile([128, 1024], BF, tag="est", name="est")
                    if w == TQB:
                        nc.scalar.activation(est[:, 0:1024], pst[:, 0:1024],
                                             mybir.ActivationFunctionType.Exp,
                                             scale=scale)
                    else:
                        for h in range(HPC):
                            nc.scalar.activation(
                                est[:, h * 512:h * 512 + w],
                                pst[:, h * 512:h * 512 + w],
                                mybir.ActivationFunctionType.Exp, scale=scale)
                    if 128 * j >= TQB * i:  # diagonal tile: mask upper part
                        for h in range(HPC):
                            nc.gpsimd.affine_select(
                                out=est[:, h * 512:h * 512 + 128],
                                in_=est[:, h * 512:h * 512 + 128],
                                compare_op=mybir.AluOpType.is_ge,
                                fill=0.0, base=0, pattern=[[1, 128]],
                                channel_multiplier=-1)
                    ests[j] = est

                def av_unit(j):
                    w, lo = widths[j]
                    est = ests[j]
                    s_min = max(0, j - JPB * i)
                    for h in range(HPC):
                        for s in range(s_min, JPB):
                            local = TQB * i + 128 * s - lo
                            g = pocol(h, s)
                            # one start=True per PSUM bank per round (lazy
                            # bank-granular zeroing); s>0 first-writes land
                            # on pending-zero bytes and write fresh.
                            nc.tensor.matmul(
                                po[:, g:g + C + 1],
                                est[:, h * 512 + local:h * 512 + local + 128],
                                v2_t[j][:, h * (C + 1):(h + 1) * (C + 1)],
                                start=(j == 0 and s == 0),
                                stop=(j == JPB * i + s),
                                skip_group_check=True)

                def tail_a(s):
                    """reciprocal + normalize-evict: frees po group s."""
                    for h in range(HPC):
                        g = pocol(h, s)
                        rd = prd.tile([128, 1], F32, tag="rd", name="rd")
                        nc.vector.reciprocal(rd[:], po[:, g + C:g + C + 1])
                        on = pon.tile([128, C], BF, tag="on", name="on")
                        nc.vector.tensor_scalar_mul(on[:], po[:, g:g + C],
                                                    rd[:])
                        ons[(h, s)] = on

                def tail_b(s):
                    def f():
                        otT = potT.tile([128, 128], BF, tag="otT", name="otT")
                        # pp doubles as transpose scratch (cols 0:128 f32)
                        # before the proj matmuls overwrite it bank-wide.
                        pp = pstp.tile([128, 1024], F32, tag="pst", name="pp")
                        for h in range(HPC):
                            on = ons[(h, s)]
                            scrb = pp[0:C, h * 64:(h + 1) * 64].bitcast(BF)
                            nc.tensor.transpose(scrb, on[:], ident[:])
                            nc.vector.tensor_copy(otT[h * C:(h + 1) * C, :],
                                                  scrb)
                        for g2 in range(2):
                            nc.tensor.matmul(
                                pp[:, g2 * 512:(g2 + 1) * 512], otT[:],
                                wp_ap[:, g2 * 512:(g2 + 1) * 512],
                                start=True, stop=True, skip_group_check=True)
                        osb = posb.tile([128, D], BF, tag="osb", name="osb")
                        nc.vector.tensor_copy(osb[:], pp[:])
                        tt = i * JPB + s
                        nc.sync.dma_start(out_d[tt * 128:(tt + 1) * 128, :],
                                          osb[:])
                    return f

                work = list(deferred) + list(filler)
                popped = 0

                def pop_work(upto):
                    nonlocal popped
                    while popped < min(upto, len(work)):
                        work[popped]()
                        popped += 1

                tail_bs = []

                def post_av(j):
                    if j >= JPB * i:
                        s = j - JPB * i
                        tail_a(s)
                        tail_bs.append(tail_b(s))

                for j in range(njt):
                    sc_unit(j)
                    pop_work((j + 1) * len(work) // njt)
                    if j >= LAG:
                        av_unit(j - LAG)
                        post_av(j - LAG)
                for j in range(max(0, njt - LAG), njt):
                    av_unit(j)
                    post_av(j)
                pop_work(len(work))
                if i == NI - 1:
                    for f in tail_bs:
                        f()
                    tail_bs = []
                return tail_bs

            # ---------------- main loop ----------------
            for u in b_units(0):
                u()
            deferred = []
            for i in range(NI):
                filler = b_units(i + 1) if i + 1 < NI else []
                deferred = c_block(i, filler, deferred)

            if dump:
                for i in range(NI):
                    nc.sync.dma_start(qr_dump[:, i * TQB:(i + 1) * TQB],
                                      qr_t[i][:])
                    nc.sync.dma_start(kr_dump[:, i * TQB:(i + 1) * TQB],
                                      kr_t[i][:])
                for j in range(NT):
                    nc.sync.dma_start(v_dump[j * 128:(j + 1) * 128, :],
                                      v2_t[j][:])

    nc.compile()
    return nc


def host_inputs(x, W_qkv, W_proj, n_cores=N_CORES):
    """Shard full inputs into per-core packed input maps (bf16)."""
    x = np.asarray(x, np.float32)
    W_qkv = np.asarray(W_qkv, np.float32)
    W_proj = np.asarray(W_proj, np.float32)
    T, D = x.shape
    C = C_HEAD
    H = D // C
    HPC = H // n_cores
    C2 = HPC * C
    ND = D // 128
    TQB = 512
    NI = T // TQB
    bf = ml_dtypes.bfloat16
    Wq, Wk, Wv = W_qkv[0:D], W_qkv[D:2 * D], W_qkv[2 * D:3 * D]

    # xH[p, i, d, t] = x[i*TQB+t, d*128+p]
    xH = np.ascontiguousarray(
        x.reshape(NI, TQB, ND, 128).transpose(3, 0, 2, 1)
        .reshape(128, NI * ND * TQB)).astype(bf)

    inv_freq = 1.0 / (10000.0 ** (np.arange(0, C, 2, dtype=np.float64) / C))
    ang = np.arange(T, dtype=np.float64)[None, :] * \
        np.repeat(inv_freq, 2)[:, None]          # [C, T]
    cosT = np.tile(np.cos(ang), (HPC, 1))        # [C2, T]
    sinT = np.tile(np.sin(ang), (HPC, 1))
    # csH[p, i, {cos,sin}, t]
    csH = np.stack([cosT.reshape(C2, NI, TQB), sinT.reshape(C2, NI, TQB)],
                   axis=2).reshape(C2, NI * 2 * TQB)
    csH = np.ascontiguousarray(csH).astype(bf)

    perm = np.zeros((C2, C2), np.float32)
    for cp in range(C2):
        if cp % 2 == 0:
            perm[cp + 1, cp] = -1.0
        else:
            perm[cp - 1, cp] = 1.0

    in_maps = []
    for c in range(n_cores):
        rows = slice(c * C2, (c + 1) * C2)
        # wH[p, d, {q,k,v}, c2] = W_*[c*C2+c2, d*128+p]
        wH = np.stack([Wq[rows].T.reshape(ND, 128, C2),
                       Wk[rows].T.reshape(ND, 128, C2),
                       Wv[rows].T.reshape(ND, 128, C2)],
                      axis=1)                     # [ND, 3, 128, C2]
        wH = np.ascontiguousarray(
            wH.transpose(2, 0, 1, 3).reshape(128, ND * 3 * C2)).astype(bf)
        pwH = np.concatenate(
            [perm, W_proj[:, c * C2:(c + 1) * C2].T], axis=1).astype(bf)
        in_maps.append({
            "xH": xH,
            "wH": wH,
            "csH": csH,
            "pwH": np.ascontiguousarray(pwH),
        })
    return in_maps


_PROGRAM_CACHE = {}


def _get_program(T, D, use_f32r=True):
    key = (T, D)
    if key not in _PROGRAM_CACHE:
        _PROGRAM_CACHE[key] = build_program(T=T, D=D)
    return _PROGRAM_CACHE[key]


def run_cores(x, W_qkv, W_proj, **run_kwargs):
    nc = _get_program(x.shape[0], x.shape[1])
    in_maps = host_inputs(x, W_qkv, W_proj)
    return run_bass_kernel_spmd(nc, in_maps, core_ids=list(range(N_CORES)),
                                **run_kwargs)


def kernel(x, W_qkv, W_proj):
    res = run_cores(x, W_qkv, W_proj)
    out = np.zeros((x.shape[0], x.shape[1]), np.float32)
    for r in res.results:
        out += np.asarray(r["out"], np.float32)
    return out


# revision 21
# speedup vs baseline: 1.2390x; 1.0099x over previous
"""Trainium2 Bass kernel for causal self-attention (GPT-J RoPE), 8-way
tensor-parallel over heads.

Contract: kernel(x, W_qkv, W_proj) -> np.ndarray  (full [T, D] output)

Sharding: 16 heads / 8 cores = 2 heads per core. Each core computes its
2 heads' QKV projection, RoPE, causal attention, and its partial
W_proj contribution; the host sums the 8 partial outputs (the TP
all-reduce), which is the unshard step.

v3 design (single fused loop, all matmul operands bf16):
  - Per query block i (512 queries): QKV+RoPE+V-transpose work for
    block i+1 is interleaved as PE filler into block i's attention
    j-loop, so the PE stays busy while the Activation engine (the
    softmax-exp bottleneck) streams. AV matmuls lag the score/exp
    stream by 2 key tiles so they never wait on the exp.
  - Scores per key tile j: one PSUM tile [128, 1024] holds both heads
    (h0 at cols 0:w, h1 at 512:512+w); one/two exp activations emit
    est (bf16, SBUF); diagonal masking via gpsimd affine_select.
  - AV transposed: po[tq, 65] += est_slice^T @ (v | ones), 65-wide
    bf16 matmuls (half the PE cost of the [c, tq] orientation); the
    ones column yields the softmax denominator for free. Exactly one
    start=True per PSUM bank per round (PSUM zeroing is lazy at 2KB
    bank granularity).
  - Tails split: tailA (reciprocal + normalize-evict, frees po early)
    runs inline; tailB (PE transpose, head-stacked K=128 W_proj
    matmul, eviction, output DMA) is deferred into the next block's
    j-loop so its serial cross-engine chain hides under the exp
    stream.
  - Host packs x/weights/cos-sin/perm+wp into 4 DRAM tensors laid out
    so each block needs a single input DMA (50 DMAs total/core).
  - Engines: ACT = exp only; DVE = PSUM evictions + PSUM-reading
    muls; Pool = SBUF-only muls/adds/masks; PE = matmuls.
"""

import math
import sys

if "/opt/trn_rl_repo" not in sys.path:
    sys.path.insert(0, "/opt/trn_rl_repo")

import numpy as np
import ml_dtypes

import concourse.bass as bass  # noqa: F401
import concourse.mybir as mybir
import concourse.tile as tile
from concourse import bacc
from concourse.bass_utils import run_bass_kernel_spmd
from concourse.masks import make_identity

F32 = mybir.dt.float32
BF = mybir.dt.bfloat16

N_CORES = 8
N_HEAD = 16
T_FULL = 4096
D_FULL = 1024
C_HEAD = 64
LAG = 2


def build_program(T=4096, D=1024, C=64, num_devices=8, dump=False):
    HPC = 2
    C2 = HPC * C            # 128
    TQB = 512
    ND = D // 128           # 8 d-tiles
    NT = T // 128           # 32 key tiles
    NI = T // TQB           # 8 query blocks
    JPB = TQB // 128        # 4

    scale = 1.0 / math.sqrt(C)

    nc = bacc.Bacc(
        "TRN2",
        target_bir_lowering=False,
        debug=False,
        enable_asserts=False,
        num_devices=num_devices,
    )

    # xH[p, i*ND*TQB + d*TQB + t] = x[i*TQB+t, d*128+p]
    xH_d = nc.dram_tensor("xH", [128, T * ND], BF, kind="ExternalInput").ap()
    # wH[p, d*3*C2 + {q,k,v}*C2 + c] = W_{q,k,v}[core_c2_c, d*128+p]
    wH_d = nc.dram_tensor("wH", [128, ND * 3 * C2], BF,
                          kind="ExternalInput").ap()
    # csH[p, i*2*TQB + {cos,sin}*TQB + t]
    csH_d = nc.dram_tensor("csH", [C2, NI * 2 * TQB], BF,
                           kind="ExternalInput").ap()
    # pwH = [perm | wp]  ([C2, C2 + D])
    pwH_d = nc.dram_tensor("pwH", [C2, C2 + D], BF, kind="ExternalInput").ap()
    out_d = nc.dram_tensor("out", [T, D], BF, kind="ExternalOutput").ap()
    if dump:
        qr_dump = nc.dram_tensor("qr_dump", [C2, T], BF,
                                 kind="ExternalOutput").ap()
        kr_dump = nc.dram_tensor("kr_dump", [C2, T], BF,
                                 kind="ExternalOutput").ap()
        v_dump = nc.dram_tensor("v_dump", [T, 2 * (C + 1)], BF,
                                kind="ExternalOutput").ap()

    with tile.TileContext(nc) as tc:
        with (
            tc.tile_pool(name="const", bufs=1) as pconst,
            tc.tile_pool(name="qk", bufs=1) as pqk,
            tc.tile_pool(name="v2", bufs=1) as pv2,
            tc.tile_pool(name="xs", bufs=3) as px,
            tc.tile_pool(name="cs", bufs=2) as pcs,
            tc.tile_pool(name="raw", bufs=3) as praw,
            tc.tile_pool(name="est", bufs=5) as pest,
            tc.tile_pool(name="on", bufs=10) as pon,
            tc.tile_pool(name="rd", bufs=10) as prd,
            tc.tile_pool(name="otT", bufs=4) as potT,
            tc.tile_pool(name="osb", bufs=4) as posb,
            tc.tile_pool(name="pst", bufs=2, space="PSUM") as pstp,
            tc.tile_pool(name="po", bufs=1, space="PSUM") as ppo,
            tc.tile_pool(name="bps", bufs=1, space="PSUM") as pbps,
        ):
            # ---------------- constants ----------------
            # (weight DMAs are emitted after B(0)'s input DMA, per-d, so the
            # first qkv matmuls start ~1.5us in instead of ~12us)
            wd_sb = [pconst.tile([128, 3 * C2], BF, tag=f"wd{d}",
                                 name=f"wd{d}") for d in range(ND)]
            pw_sb = pconst.tile([C2, C2 + D], BF, tag="pw", name="pw_sb")

            def emit_const_dmas():
                for d in range(ND):
                    nc.sync.dma_start(
                        wd_sb[d][:], wH_d[:, d * 3 * C2:(d + 1) * 3 * C2])
                nc.sync.dma_start(pw_sb[:], pwH_d[:])

            def w_ap(d, which):   # which: 0=q 1=k 2=v -> [128, C2] lhsT
                return wd_sb[d][:, which * C2:(which + 1) * C2]

            perm_ap = pw_sb[:, 0:C2]
            wp_ap = pw_sb[:, C2:C2 + D]

            ident = pconst.tile([128, 128], BF, tag="ident", name="ident")
            make_identity(nc, ident[:])

            # persistent rope'd q/k [c2, 512] per block, v tiles [t, 130]
            qr_t = [pqk.tile([C2, TQB], BF, tag=f"qr{i}", name=f"qr{i}")
                    for i in range(NI)]
            kr_t = [pqk.tile([C2, TQB], BF, tag=f"kr{i}", name=f"kr{i}")
                    for i in range(NI)]
            v2_t = [pv2.tile([128, 2 * (C + 1)], BF, tag=f"v{j}", name=f"v{j}")
                    for j in range(NT)]
            for j in range(NT):
                nc.gpsimd.memset(v2_t[j][:, C:C + 1], 1.0)
                nc.gpsimd.memset(v2_t[j][:, 2 * C + 1:2 * C + 2], 1.0)

            # ---------------- B(i): qkv + rope + vT units ----------------
            def b_units(ib):
                """Thunk list computing qr[ib], kr[ib], v2[4ib..4ib+3]."""
                st = {}
                units = []   # (pe_cost_ns, thunk)

                def u_dma():
                    st["xt"] = px.tile([128, ND * TQB], BF, tag="xt",
                                       name="xt")
                    nc.sync.dma_start(
                        st["xt"][:],
                        xH_d[:, ib * ND * TQB:(ib + 1) * ND * TQB])
                    st["cs"] = pcs.tile([C2, 2 * TQB], BF, tag="cs",
                                        name="cs")
                    nc.sync.dma_start(
                        st["cs"][:],
                        csH_d[:, ib * 2 * TQB:(ib + 1) * 2 * TQB])
                    st["qk"] = pbps.tile([128, 1024], F32, tag="bps",
                                         name="bqk")
                units.append((0, u_dma))

                def u_qk(d):
                    def f():
                        xs = st["xt"][:, d * TQB:(d + 1) * TQB]
                        nc.tensor.matmul(st["qk"][:, 0:TQB], w_ap(d, 0), xs,
                                         start=(d == 0), stop=(d == ND - 1),
                                         skip_group_check=True)
                        nc.tensor.matmul(st["qk"][:, TQB:2 * TQB], w_ap(d, 1),
                                         xs, start=(d == 0),
                                         stop=(d == ND - 1),
                                         skip_group_check=True)
                    return f
                for d in range(ND):
                    units.append((430, u_qk(d)))

                def u_evqk():
                    st["rawq"] = praw.tile([C2, TQB], BF, tag="raw",
                                           name="rawq")
                    nc.vector.tensor_copy(st["rawq"][:], st["qk"][:, 0:TQB])
                    st["rawk"] = praw.tile([C2, TQB], BF, tag="raw",
                                           name="rawk")
                    nc.vector.tensor_copy(st["rawk"][:],
                                          st["qk"][:, TQB:2 * TQB])
                    st["rot"] = pbps.tile([128, 1024], F32, tag="bps",
                                          name="brot")
                units.append((0, u_evqk))

                def u_rope(which):
                    def f():
                        raw = st["rawq"] if which == 0 else st["rawk"]
                        dst = qr_t[ib] if which == 0 else kr_t[ib]
                        pr = st["rot"][:, which * TQB:(which + 1) * TQB]
                        nc.tensor.matmul(pr, perm_ap, raw[:],
                                         start=True, stop=True,
                                         skip_group_check=True)
                        qc = praw.tile([C2, TQB], BF, tag="qc", name="qc")
                        nc.gpsimd.tensor_mul(qc[:], raw[:],
                                             st["cs"][:, 0:TQB])
                        qs = praw.tile([C2, TQB], BF, tag="qs", name="qs")
                        nc.vector.tensor_mul(qs[:], pr,
                                             st["cs"][:, TQB:2 * TQB])
                        nc.gpsimd.tensor_add(dst[:], qc[:], qs[:])
                    return f
                units.append((220, u_rope(0)))
                units.append((220, u_rope(1)))

                def u_valloc():
                    st["pv"] = pbps.tile([128, 1024], F32, tag="bps",
                                         name="bpv")
                units.append((0, u_valloc))

                def u_v(d):
                    def f():
                        nc.tensor.matmul(
                            st["pv"][:, 0:TQB], w_ap(d, 2),
                            st["xt"][:, d * TQB:(d + 1) * TQB],
                            start=(d == 0), stop=(d == ND - 1),
                            skip_group_check=True)
                    return f
                for d in range(ND):
                    units.append((215, u_v(d)))

                def u_evv():
                    st["vraw"] = praw.tile([C2, TQB], BF, tag="raw",
                                           name="vraw")
                    nc.vector.tensor_copy(st["vraw"][:], st["pv"][:, 0:TQB])
                    st["pvt"] = pbps.tile([128, 1024], F32, tag="bps",
                                          name="bpvt")
                units.append((0, u_evv))

                def u_vt(s):
                    def f():
                        pvtb = st["pvt"][:, s * 64:(s + 1) * 64].bitcast(BF)
                        nc.tensor.transpose(
                            pvtb, st["vraw"][:, s * 128:(s + 1) * 128],
                            ident[:])
                        j = ib * JPB + s
                        nc.vector.tensor_copy(v2_t[j][:, 0:C], pvtb[:, 0:C])
                        nc.vector.tensor_copy(v2_t[j][:, C + 1:2 * C + 1],
                                              pvtb[:, C:2 * C])
                    return f
                for s in range(JPB):
                    units.append((55, u_vt(s)))

                return units

            # ---------------- C(i) ----------------
            def pocol(h, s):
                return 512 * h + 65 * s

            def c_block(i, filler, deferred):
                """Emit attention block i. `filler`: B(i+1) units; `deferred`:
                tailB units of block i-1. Returns this block's tailB units
                (empty if emitted inline for the last block)."""
                njt = (i + 1) * JPB
                po = ppo.tile([128, 1024], F32, tag="po", name="po")
                ests = {}
                widths = {}
                ons = {}

                def sc_unit(j):
                    w = min(TQB, TQB * (i + 1) - 128 * j)
                    lo = max(TQB * i, 128 * j)
                    widths[j] = (w, lo)
                    jc, jo = divmod(j, JPB)
                    pst = pstp.tile([128, 1024], F32, tag="pst", name="pst")
                    for h in range(HPC):
                        klhs = kr_t[jc][h * C:(h + 1) * C,
                                        jo * 128:(jo + 1) * 128]
                        qrhs = qr_t[i][h * C:(h + 1) * C,
                                       lo - TQB * i:lo - TQB * i + w]
                        nc.tensor.matmul(pst[:, h * 512:h * 512 + w],
                                         klhs, qrhs, start=True, stop=True,
                                         skip_group_check=True)
                    est = pest.tile([128, 1024], BF, tag="est", name="est")
                    if w == TQB:
                        nc.scalar.activation(est[:, 0:1024], pst[:, 0:1024],
                                             mybir.ActivationFunctionType.Exp,
                                             scale=scale)
                    else:
                        for h in range(HPC):
                            nc.scalar.activation(
                                est[:, h * 512:h * 512 + w],
                                pst[:, h * 512:h * 512 + w],
                                mybir.ActivationFunctionType.Exp, scale=scale)
                    if 128 * j >= TQB * i:  # diagonal tile: mask upper part
                        for h in range(HPC):
                            nc.gpsimd.affine_select(
                                out=est[:, h * 512:h * 512 + 128],
                                in_=est[:, h * 512:h * 512 + 128],
                                compare_op=mybir.AluOpType.is_ge,
                                fill=0.0, base=0, pattern=[[1, 128]],
                                channel_multiplier=-1)
                    ests[j] = est

                def av_unit(j):
                    w, lo = widths[j]
                    est = ests[j]
                    s_min = max(0, j - JPB * i)
                    for h in range(HPC):
                        for s in range(s_min, JPB):
                            local = TQB * i + 128 * s - lo
                            g = pocol(h, s)
                            # one start=True per PSUM bank per round (lazy
                            # bank-granular zeroing); s>0 first-writes land
                            # on pending-zero bytes and write fresh.
                            nc.tensor.matmul(
                                po[:, g:g + C + 1],
                                est[:, h * 512 + local:h * 512 + local + 128],
                                v2_t[j][:, h * (C + 1):(h + 1) * (C + 1)],
                                start=(j == 0 and s == 0),
                                stop=(j == JPB * i + s),
                                skip_group_check=True)

                def tail_a(s):
                    """reciprocal + normalize-evict: frees po group s."""
                    for h in range(HPC):
                        g = pocol(h, s)
                        rd = prd.tile([128, 1], F32, tag="rd", name="rd")
                        nc.vector.reciprocal(rd[:], po[:, g + C:g + C + 1])
                        on = pon.tile([128, C], BF, tag="on", name="on")
                        nc.vector.tensor_scalar_mul(on[:], po[:, g:g + C],
                                                    rd[:])
                        ons[(h, s)] = on

                def tail_b(s):
                    def f():
                        otT = potT.tile([128, 128], BF, tag="otT", name="otT")
                        # pp doubles as transpose scratch (cols 0:128 f32)
                        # before the proj matmuls overwrite it bank-wide.
                        pp = pstp.tile([128, 1024], F32, tag="pst", name="pp")
                        for h in range(HPC):
                            on = ons[(h, s)]
                            scrb = pp[0:C, h * 64:(h + 1) * 64].bitcast(BF)
                            nc.tensor.transpose(scrb, on[:], ident[:])
                            nc.vector.tensor_copy(otT[h * C:(h + 1) * C, :],
                                                  scrb)
                        for g2 in range(2):
                            nc.tensor.matmul(
                                pp[:, g2 * 512:(g2 + 1) * 512], otT[:],
                                wp_ap[:, g2 * 512:(g2 + 1) * 512],
                                start=True, stop=True, skip_group_check=True)
                        osb = posb.tile([128, D], BF, tag="osb", name="osb")
                        nc.vector.tensor_copy(osb[:], pp[:])
                        tt = i * JPB + s
                        nc.sync.dma_start(out_d[tt * 128:(tt + 1) * 128, :],
                                          osb[:])
                    return f

                work = list(deferred) + list(filler)
                popped = 0

                def pop_budget(budget):
                    nonlocal popped
                    while popped < len(work) and budget > 0:
                        cost, f = work[popped]
                        f()
                        budget -= max(cost, 60)
                        popped += 1

                def pop_all():
                    nonlocal popped
                    while popped < len(work):
                        work[popped][1]()
                        popped += 1

                tail_bs = []

                def av_cost(j):
                    s_min = max(0, j - JPB * i)
                    return HPC * (JPB - s_min) * 30

                def post_av(j):
                    if j >= JPB * i:
                        s = j - JPB * i
                        tail_a(s)
                        if i == NI - 1:
                            tail_bs.append((s, tail_b(s)))
                        else:
                            tail_bs.append((s, tail_b(s)))

                for j in range(njt):
                    sc_unit(j)
                    w = widths[j][0]
                    period = (2 * w + 222) * 0.833
                    budget = period - 430 - (av_cost(j - LAG) if j >= LAG
                                             else 0)
                    if j >= LAG:
                        av_unit(j - LAG)
                        post_av(j - LAG)
                    pop_budget(budget)
                    # last block: emit ready tailBs inline, two js after
                    # their tailA, so the final drain is short
                    if i == NI - 1 and tail_bs and j - LAG - JPB * i >=                             tail_bs[0][0] + LAG:
                        tail_bs.pop(0)[1]()
                for j in range(max(0, njt - LAG), njt):
                    av_unit(j)
                    post_av(j)
                pop_all()
                if i == NI - 1:
                    for _, f in tail_bs:
                        f()
                    tail_bs = []
                return [(530, f) for _, f in tail_bs]

            # ---------------- main loop ----------------
            b0 = b_units(0)
            b0[0][1]()            # B(0) input DMA first
            emit_const_dmas()     # then weights (per-d) + perm/wp
            for _, u in b0[1:]:
                u()
            deferred = []
            for i in range(NI):
                filler = b_units(i + 1) if i + 1 < NI else []
                deferred = c_block(i, filler, deferred)

            if dump:
                for i in range(NI):
                    nc.sync.dma_start(qr_dump[:, i * TQB:(i + 1) * TQB],
                                      qr_t[i][:])
                    nc.sync.dma_start(kr_dump[:, i * TQB:(i + 1) * TQB],
                                      kr_t[i][:])
                for j in range(NT):
                    nc.sync.dma_start(v_dump[j * 128:(j + 1) * 128, :],
                                      v2_t[j][:])

    nc.compile()
    return nc


def host_inputs(x, W_qkv, W_proj, n_cores=N_CORES):
    """Shard full inputs into per-core packed input maps (bf16)."""
    x = np.asarray(x, np.float32)
    W_qkv = np.asarray(W_qkv, np.float32)
    W_proj = np.asarray(W_proj, np.float32)
    T, D = x.shape
    C = C_HEAD
    H = D // C
    HPC = H // n_cores
    C2 = HPC * C
    ND = D // 128
    TQB = 512
    NI = T // TQB
    bf = ml_dtypes.bfloat16
    Wq, Wk, Wv = W_qkv[0:D], W_qkv[D:2 * D], W_qkv[2 * D:3 * D]

    # xH[p, i, d, t] = x[i*TQB+t, d*128+p]
    xH = np.ascontiguousarray(
        x.reshape(NI, TQB, ND, 128).transpose(3, 0, 2, 1)
        .reshape(128, NI * ND * TQB)).astype(bf)

    inv_freq = 1.0 / (10000.0 ** (np.arange(0, C, 2, dtype=np.float64) / C))
    ang = np.arange(T, dtype=np.float64)[None, :] * \
        np.repeat(inv_freq, 2)[:, None]          # [C, T]
    cosT = np.tile(np.cos(ang), (HPC, 1))        # [C2, T]
    sinT = np.tile(np.sin(ang), (HPC, 1))
    # csH[p, i, {cos,sin}, t]
    csH = np.stack([cosT.reshape(C2, NI, TQB), sinT.reshape(C2, NI, TQB)],
                   axis=2).reshape(C2, NI * 2 * TQB)
    csH = np.ascontiguousarray(csH).astype(bf)

    perm = np.zeros((C2, C2), np.float32)
    for cp in range(C2):
        if cp % 2 == 0:
            perm[cp + 1, cp] = -1.0
        else:
            perm[cp - 1, cp] = 1.0

    in_maps = []
    for c in range(n_cores):
        rows = slice(c * C2, (c + 1) * C2)
        # wH[p, d, {q,k,v}, c2] = W_*[c*C2+c2, d*128+p]
        wH = np.stack([Wq[rows].T.reshape(ND, 128, C2),
                       Wk[rows].T.reshape(ND, 128, C2),
                       Wv[rows].T.reshape(ND, 128, C2)],
                      axis=1)                     # [ND, 3, 128, C2]
        wH = np.ascontiguousarray(
            wH.transpose(2, 0, 1, 3).reshape(128, ND * 3 * C2)).astype(bf)
        pwH = np.concatenate(
            [perm, W_proj[:, c * C2:(c + 1) * C2].T], axis=1).astype(bf)
        in_maps.append({
            "xH": xH,
            "wH": wH,
            "csH": csH,
            "pwH": np.ascontiguousarray(pwH),
        })
    return in_maps


_PROGRAM_CACHE = {}


def _get_program(T, D, use_f32r=True):
    key = (T, D)
    if key not in _PROGRAM_CACHE:
        _PROGRAM_CACHE[key] = build_program(T=T, D=D)
    return _PROGRAM_CACHE[key]


def run_cores(x, W_qkv, W_proj, **run_kwargs):
    nc = _get_program(x.shape[0], x.shape[1])
    in_maps = host_inputs(x, W_qkv, W_proj)
    return run_bass_kernel_spmd(nc, in_maps, core_ids=list(range(N_CORES)),
                                **run_kwargs)


def kernel(x, W_qkv, W_proj):
    res = run_cores(x, W_qkv, W_proj)
    out = np.zeros((x.shape[0], x.shape[1]), np.float32)
    for r in res.results:
        out += np.asarray(r["out"], np.float32)
    return out
